# revision 9
# baseline (speedup 1.0000x reference)
"""Trainium2 Bass kernel for the ExpCloudMMD loss.

reference math (gamma = 0.5):
  t1 = mean_{j,k} exp(-g*||p_j - p_k||^2)            over [8192, 8192]
  t2 = 2/(Nx*Np) * sum_{i,j} exp(-g*||x_i - p_j||^2) over [32768, 8192]
  out = t1 - t2  (f32 scalar)

Strategy (8 cores, SPMD, no collectives):
  - t2: shard x rows 8-way; each core computes its 4096x8192 cross block.
  - t1: the particle Gram is symmetric; in 2048x2048 super-blocks only the
    diagonal (4) + strict upper (6) of the 4x4 grid are computed, and the
    host doubles the upper sums. The 160 (row-block, col-group) pairs are
    dealt round-robin to the 8 cores; each core's pair list rides in its
    `pslhs` input tensor, so the program stays identical across cores.
  - The exp *argument* p.x - g|x|^2 - g|p|^2 is produced directly by a
    single K=68 matmul per output tile using an augmented bf16 hi/lo
    encoding (4-way split product + norm channels), so ScalarE needs no
    bias and the pipeline per [128, 2048] PSUM group is:
        4x matmul (PE) -> 1x activation(Exp, accum_out) (ACT)
    ACT is the roofline engine (~1 elem/lane/cycle @ 1.2 GHz).
  - Each ACT writes its partial row-sums into one column of a [128, 148]
    SBUF accumulator; the accumulator is DMA'd out and the final (tiny)
    weighted reduction + scaling happens on the host in float64.
"""

import threading

import ml_dtypes
import numpy as np

import concourse.bass as bass  # noqa: F401
import concourse.mybir as mybir
import concourse.tile as tile
from concourse import bacc, bass_utils

bf16 = ml_dtypes.bfloat16

GAMMA = 0.5
NX, NP, D = 32768, 8192, 16
N_CORES = 8
XS = NX // N_CORES     # 4096 x rows per core
K = 68                 # 4*16 (hi/lo product blocks) + 2 + 2 norm channels

# t1 coarse-triangle schedule: for col-super-group g (2048 particles),
# the computed row-blocks are the 16*(g+1) blocks of super-rows 0..g,
# dealt round-robin (r % 8) to cores -> per-core counts 2,4,6,8.
T1_COUNTS = [2, 4, 6, 8]
N_T1_PAIRS = sum(T1_COUNTS)                    # 20 per core
PS_COLS = N_T1_PAIRS * 128                     # 2560 pslhs columns per core
GC = 2048                                      # PSUM group columns


def _n_groups(gc):
    n_cross = (NP // 128) * (XS // gc)
    n_t1 = N_T1_PAIRS * (2048 // gc)
    return n_cross, n_cross + n_t1

N_PCHUNK = 8  # plhs load chunks (8 j-blocks each) for early compute start


def _t1_pairs(core):
    """[(row_block, col_group, weight)] for this core, in program order."""
    pairs = []
    for g in range(4):
        rows = [r for r in range(16 * (g + 1)) if r % N_CORES == core]
        assert len(rows) == T1_COUNTS[g]
        for r in rows:
            pairs.append((r, g, 1.0 if r // 16 == g else 2.0))
    return pairs


def _build_nc(repeats=1, gc=None, inplace=False):
    if gc is None:
        gc = GC
    n_cross_groups, n_groups = _n_groups(gc)
    n_bufs = 4096 // gc  # PSUM: 8 banks x 512 f32 per partition
    nc = bacc.Bacc(
        "TRN2",
        target_bir_lowering=False,
        debug=False,
        enable_asserts=False,
        num_devices=N_CORES,
    )
    dt = mybir.dt
    plhs = nc.dram_tensor("plhs", [K, NP], dt.bfloat16, kind="ExternalInput").ap()
    prhs = nc.dram_tensor("prhs", [K, NP], dt.bfloat16, kind="ExternalInput").ap()
    xrhs = nc.dram_tensor("xrhs", [K, XS], dt.bfloat16, kind="ExternalInput").ap()
    pslhs = nc.dram_tensor("pslhs", [K, PS_COLS], dt.bfloat16, kind="ExternalInput").ap()
    acc_d = nc.dram_tensor("acc", [128, n_groups], dt.float32, kind="ExternalOutput").ap()

    with tile.TileContext(nc) as tc:
        with (
            tc.tile_pool(name="const", bufs=1) as const,
            tc.tile_pool(name="scrp", bufs=2) as scrp,
            tc.tile_pool(name="psp", bufs=n_bufs, space="PSUM") as psp,
        ):
            sb_plhs = const.tile([K, NP], dt.bfloat16)
            sb_prhs = const.tile([K, NP], dt.bfloat16)
            sb_xrhs = const.tile([K, XS], dt.bfloat16)
            sb_pslhs = const.tile([K, PS_COLS], dt.bfloat16)
            sb_acc = const.tile([128, n_groups], dt.float32)
            sb_tiny = const.tile([1, 1], dt.float32)

            # Warm the ACT exp table set (~2.7us) during the DMA prologue.
            nc.gpsimd.memset(sb_tiny[:], 0.0)
            nc.scalar.activation(
                sb_tiny[:], sb_tiny[:], mybir.ActivationFunctionType.Exp
            )

            # Input loads, in consumption order. The first matmul only
            # needs plhs chunk 0 + the first xrhs half.
            pchunk = NP // N_PCHUNK
            nc.sync.dma_start(sb_plhs[:, 0:pchunk], plhs[:, 0:pchunk])
            nc.sync.dma_start(sb_xrhs[:, 0:2048], xrhs[:, 0:2048])
            nc.sync.dma_start(sb_xrhs[:, 2048:XS], xrhs[:, 2048:XS])
            for i in range(1, N_PCHUNK):
                s = slice(i * pchunk, (i + 1) * pchunk)
                nc.sync.dma_start(sb_plhs[:, s], plhs[:, s])
            nc.sync.dma_start(sb_pslhs[:], pslhs[:])
            nc.sync.dma_start(sb_prhs[:], prhs[:])

            col = 0

            def group(lhs_tile, j, rhs_tile, g):
                """One [128, gc] output group: gc/512 matmuls + fused exp-rowsum."""
                nonlocal col
                ps_t = psp.tile([128, gc], dt.float32, tag="ps")
                for q in range(gc // 512):
                    nc.tensor.matmul(
                        ps_t[:, q * 512:(q + 1) * 512],
                        lhs_tile[:, j * 128:(j + 1) * 128],
                        rhs_tile[:, g * gc + q * 512: g * gc + (q + 1) * 512],
                    )
                if inplace:
                    out_ap = ps_t[:]
                else:
                    scr = scrp.tile([128, gc], dt.float32, tag="scr")
                    out_ap = scr[:]
                nc.scalar.activation(
                    out_ap,
                    ps_t[:],
                    mybir.ActivationFunctionType.Exp,
                    accum_out=sb_acc[:, col:col + 1],
                )
                col += 1

            if repeats == 0:  # timing-only baseline: I/O but no compute
                nc.gpsimd.memset(sb_acc[:], 0.0)
            for _ in range(repeats):  # repeats>1 is a timing-only variant
                col = 0
                # t2 cross part: 64 particle blocks x XS/gc x-chunks
                for j in range(NP // 128):
                    for g in range(XS // gc):
                        group(sb_plhs, j, sb_xrhs, g)
                if repeats:
                    assert col == n_cross_groups
                # t1 part: 20 (row-block, col-group-2048) pairs; the
                # row-block data is packed consecutively in pslhs, so the
                # lhsT index is the running slot while the rhs col-group
                # follows T1_COUNTS. Each 2048-pair spans 2048/gc groups.
                slot = 0
                for g in range(4):
                    for _ in range(T1_COUNTS[g]):
                        for h in range(2048 // gc):
                            group(sb_pslhs, slot, sb_prhs, g * (2048 // gc) + h)
                        slot += 1
                if repeats:
                    assert col == n_groups

            nc.sync.dma_start(acc_d[:], sb_acc[:])

    nc.compile()
    return nc


def _split_hi_lo(v):
    vh = v.astype(bf16)
    vl = (v - vh.astype(np.float32)).astype(bf16)
    return vh, vl


def _enc_lhsT(p):
    """p: [n, 16] f32 -> [K, n] bf16 stationary-side encoding."""
    n = p.shape[0]
    ph, pl = _split_hi_lo(np.ascontiguousarray(p, np.float32))
    p2 = (-GAMMA * (p.astype(np.float64) ** 2).sum(-1)).astype(np.float32)
    p2h, p2l = _split_hi_lo(p2)
    out = np.empty((K, n), bf16)
    out[0:16] = ph.T
    out[16:32] = pl.T
    out[32:48] = ph.T
    out[48:64] = pl.T
    out[64] = p2h
    out[65] = p2l
    out[66] = bf16(-GAMMA)
    out[67] = bf16(-GAMMA)
    return out


def _enc_rhs(u):
    """u: [n, 16] f32 -> [K, n] bf16 moving-side encoding."""
    n = u.shape[0]
    uh, ul = _split_hi_lo(np.ascontiguousarray(u, np.float32))
    u2 = ((u.astype(np.float64) ** 2).sum(-1)).astype(np.float32)
    u2h, u2l = _split_hi_lo(u2)
    out = np.empty((K, n), bf16)
    out[0:16] = uh.T
    out[16:32] = uh.T
    out[32:48] = ul.T
    out[48:64] = ul.T
    out[64] = bf16(1.0)
    out[65] = bf16(1.0)
    out[66] = u2h
    out[67] = u2l
    return out


_lock = threading.Lock()
_cached_nc = None


def _get_nc():
    global _cached_nc
    with _lock:
        if _cached_nc is None:
            _cached_nc = _build_nc()
        return _cached_nc


def _make_in_maps(x, particles):
    plhs = _enc_lhsT(particles)
    prhs = _enc_rhs(particles)
    in_maps = []
    for c in range(N_CORES):
        pairs = _t1_pairs(c)
        pslhs = np.concatenate(
            [plhs[:, r * 128:(r + 1) * 128] for r, _, _ in pairs], axis=1
        )
        in_maps.append(
            {
                "plhs": plhs,
                "prhs": prhs,
                "xrhs": _enc_rhs(x[c * XS:(c + 1) * XS]),
                "pslhs": np.ascontiguousarray(pslhs),
            }
        )
    return in_maps


def _combine(results, gc=None):
    if gc is None:
        gc = GC
    n_cross_groups, n_groups = _n_groups(gc)
    t2_sum = 0.0
    t1_sum = 0.0
    for c, r in enumerate(results):
        acc = r["acc"].astype(np.float64)
        t2_sum += acc[:, :n_cross_groups].sum()
        w = np.array([w for _, _, w in _t1_pairs(c)
                      for _h in range(2048 // gc)], np.float64)
        t1_sum += (acc[:, n_cross_groups:].sum(axis=0) * w).sum()
    t1 = t1_sum / (float(NP) * NP)
    t2 = 2.0 * t2_sum / (float(NX) * NP)
    return np.float32(t1 - t2)


def kernel(x, particles):
    x = np.asarray(x, np.float32)
    particles = np.asarray(particles, np.float32)
    assert x.shape == (NX, D) and particles.shape == (NP, D)

    nc = _get_nc()
    in_maps = _make_in_maps(x, particles)
    res = bass_utils.run_bass_kernel_spmd(nc, in_maps, core_ids=list(range(N_CORES)))
    return _combine(res.results)


# revision 12
# speedup vs baseline: 1.0429x; 1.0429x over previous
"""Trainium2 Bass kernel for the ExpCloudMMD loss.

reference math (gamma = 0.5):
  t1 = mean_{j,k} exp(-g*||p_j - p_k||^2)            over [8192, 8192]
  t2 = 2/(Nx*Np) * sum_{i,j} exp(-g*||x_i - p_j||^2) over [32768, 8192]
  out = t1 - t2  (f32 scalar)

Strategy (8 cores, SPMD, no collectives):
  - t2: shard x rows 8-way; each core computes its 4096x8192 cross block.
  - t1: the particle Gram is symmetric; in 2048x2048 super-blocks only the
    diagonal (4) + strict upper (6) of the 4x4 grid are computed, and the
    host doubles the upper sums. The 160 (row-block, col-group) pairs are
    dealt round-robin to the 8 cores; each core's pair list rides in its
    `pslhs` input tensor, so the program stays identical across cores.
  - The exp *argument* p.x - g|x|^2 - g|p|^2 is produced directly by a
    single K=68 matmul per output tile using an augmented bf16 hi/lo
    encoding (4-way split product + norm channels), so ScalarE needs no
    bias and the pipeline per [128, 2048] PSUM group is:
        4x matmul (PE) -> 1x activation(Exp, accum_out) (ACT)
    ACT is the roofline engine (~1 elem/lane/cycle @ 1.2 GHz).
  - Each ACT writes its partial row-sums into one column of a [128, 148]
    SBUF accumulator; the accumulator is DMA'd out and the final (tiny)
    weighted reduction + scaling happens on the host in float64.
"""

import threading

import ml_dtypes
import numpy as np

import concourse.bass as bass  # noqa: F401
import concourse.mybir as mybir
import concourse.tile as tile
from concourse import bacc, bass_utils

bf16 = ml_dtypes.bfloat16

GAMMA = 0.5
NX, NP, D = 32768, 8192, 16
N_CORES = 8
XS = NX // N_CORES     # 4096 x rows per core
K = 68                 # 4*16 (hi/lo product blocks) + 2 + 2 norm channels

# t1 coarse-triangle schedule: for col-super-group g (2048 particles),
# the computed row-blocks are the 16*(g+1) blocks of super-rows 0..g,
# dealt round-robin (r % 8) to cores -> per-core counts 2,4,6,8.
T1_COUNTS = [2, 4, 6, 8]
N_T1_PAIRS = sum(T1_COUNTS)                    # 20 per core
PS_COLS = N_T1_PAIRS * 128                     # 2560 pslhs columns per core
GC = 2048                                      # PSUM group columns


def _n_groups(gc):
    n_cross = (NP // 128) * (XS // gc)
    n_t1 = N_T1_PAIRS * (2048 // gc)
    return n_cross, n_cross + n_t1

N_PCHUNK = 8  # plhs load chunks (8 j-blocks each) for early compute start


def _t1_pairs(core):
    """[(row_block, col_group, weight)] for this core, in program order."""
    pairs = []
    for g in range(4):
        rows = [r for r in range(16 * (g + 1)) if r % N_CORES == core]
        assert len(rows) == T1_COUNTS[g]
        for r in rows:
            pairs.append((r, g, 1.0 if r // 16 == g else 2.0))
    return pairs


def _build_nc(repeats=1, gc=None, inplace=False, dve_split=True):
    """dve_split=True: ScalarE handles [128,1536] groups while VectorE
    (custom exp ops) handles [128,512] groups of every cross j-block --
    two exp engines in parallel. PSUM: ACT 2x3 banks + DVE 2x1 bank."""
    if gc is None:
        gc = GC
    nc = bacc.Bacc(
        "TRN2",
        target_bir_lowering=False,
        debug=False,
        enable_asserts=False,
        num_devices=N_CORES,
    )
    dt = mybir.dt
    plhs = nc.dram_tensor("plhs", [K, NP], dt.bfloat16, kind="ExternalInput").ap()
    prhs = nc.dram_tensor("prhs", [K, NP], dt.bfloat16, kind="ExternalInput").ap()
    xrhs = nc.dram_tensor("xrhs", [K, XS], dt.bfloat16, kind="ExternalInput").ap()
    pslhs = nc.dram_tensor("pslhs", [K, PS_COLS], dt.bfloat16, kind="ExternalInput").ap()
    n_cols = len(_plan(0)) if dve_split else _n_groups(gc)[1]
    acc_d = nc.dram_tensor("acc", [128, n_cols], dt.float32, kind="ExternalOutput").ap()
    if dve_split:
        opA, opB, opC = _register_dve_exp_ops()
        ct = [float(v) for v in _EXP_CT]

    with tile.TileContext(nc) as tc:
        with (
            tc.tile_pool(name="const", bufs=1) as const,
            tc.tile_pool(name="scrp", bufs=2) as scrp,
            tc.tile_pool(name="psp", bufs=2, space="PSUM") as psp,
            tc.tile_pool(name="psd", bufs=2, space="PSUM") as psd,
            tc.tile_pool(name="h1p", bufs=2) as h1p,
            tc.tile_pool(name="stagep", bufs=2) as stagep,
            tc.tile_pool(name="scr3p", bufs=2) as scr3p,
        ):
            sb_plhs = const.tile([K, NP], dt.bfloat16)
            sb_prhs = const.tile([K, NP], dt.bfloat16)
            sb_xrhs = const.tile([K, XS], dt.bfloat16)
            sb_pslhs = const.tile([K, PS_COLS], dt.bfloat16)
            sb_acc = const.tile([128, n_cols], dt.float32)
            sb_tiny = const.tile([1, 1], dt.float32)

            # Warm the ACT exp table set (~2.7us) during the DMA prologue.
            nc.gpsimd.memset(sb_tiny[:], 0.0)
            nc.scalar.activation(
                sb_tiny[:], sb_tiny[:], mybir.ActivationFunctionType.Exp
            )

            # Input loads, in consumption order. The first matmul only
            # needs plhs chunk 0 + the first xrhs half.
            pchunk = NP // N_PCHUNK
            nc.sync.dma_start(sb_plhs[:, 0:pchunk], plhs[:, 0:pchunk])
            nc.sync.dma_start(sb_xrhs[:, 0:2048], xrhs[:, 0:2048])
            nc.sync.dma_start(sb_xrhs[:, 2048:XS], xrhs[:, 2048:XS])
            for i in range(1, N_PCHUNK):
                s = slice(i * pchunk, (i + 1) * pchunk)
                nc.sync.dma_start(sb_plhs[:, s], plhs[:, s])
            nc.sync.dma_start(sb_pslhs[:], pslhs[:])
            nc.sync.dma_start(sb_prhs[:], prhs[:])

            col = 0
            act_w = 1536 if dve_split else gc

            def act_group(lhs_tile, j, rhs_tile, cstart, width):
                """ScalarE group: width/512 matmuls + fused exp-rowsum."""
                nonlocal col
                ps_t = psp.tile([128, width], dt.float32, tag="ps")
                for q in range(width // 512):
                    nc.tensor.matmul(
                        ps_t[:, q * 512:(q + 1) * 512],
                        lhs_tile[:, j * 128:(j + 1) * 128],
                        rhs_tile[:, cstart + q * 512: cstart + (q + 1) * 512],
                    )
                scr = scrp.tile([128, width], dt.float32, tag="scr")
                nc.scalar.activation(
                    scr[:],
                    ps_t[:],
                    mybir.ActivationFunctionType.Exp,
                    accum_out=sb_acc[:, col:col + 1],
                )
                col += 1

            dve_state = {"pend": 0, "stage": None}

            def dve_group(lhs_tile, j, rhs_tile, cstart):
                """VectorE group: 1 matmul + custom exp128 chain [128,512]."""
                nonlocal col
                st = dve_state
                ps_t = psd.tile([128, 512], dt.float32, tag="pd")
                nc.tensor.matmul(
                    ps_t[:],
                    lhs_tile[:, j * 128:(j + 1) * 128],
                    rhs_tile[:, cstart:cstart + 512],
                )
                h1 = h1p.tile([128, 512], dt.float32, tag="h1")
                nc.vector._custom_dve(
                    opA, out=h1[:], in0=ps_t[:],
                    s0=ct[5], s1=ct[4], imm2=ct[3],
                )
                if st["pend"] == 0:
                    st["stage"] = stagep.tile([128, 4096], dt.float32, tag="stage", name="stage")
                p_sl = st["stage"][:, st["pend"] * 512:(st["pend"] + 1) * 512]
                nc.vector._custom_dve(
                    opB, out=p_sl, in0=h1[:], in1=ps_t[:],
                    s0=ct[2], s1=ct[1], imm2=ct[0],
                )
                st["pend"] += 1
                if st["pend"] == 8:
                    scr3 = scr3p.tile([128, 4096], dt.float32, tag="scr3")
                    nc.vector._custom_dve(
                        opC, out=scr3[:], in0=st["stage"][:],
                        s0=0.0, s1=0.0,
                        accum_out=sb_acc[:, col:col + 1],
                    )
                    col += 1
                    st["pend"] = 0

            if repeats == 0:  # timing-only baseline: I/O but no compute
                nc.gpsimd.memset(sb_acc[:], 0.0)
            for _ in range(repeats):  # repeats>1 is a timing-only variant
                col = 0
                if dve_split:
                    for j in range(NP // 128):
                        act_group(sb_plhs, j, sb_xrhs, 0, 1536)
                        act_group(sb_plhs, j, sb_xrhs, 1536, 1536)
                        dve_group(sb_plhs, j, sb_xrhs, 3072)
                        dve_group(sb_plhs, j, sb_xrhs, 3584)
                    assert dve_state["pend"] == 0
                    slot = 0
                    for g in range(4):
                        for _t in range(T1_COUNTS[g]):
                            act_group(sb_pslhs, slot, sb_prhs, g * 2048, 1536)
                            act_group(sb_pslhs, slot, sb_prhs, g * 2048 + 1536, 512)
                            slot += 1
                    if repeats:
                        assert col == n_cols, (col, n_cols)
                else:
                    for j in range(NP // 128):
                        for g in range(XS // gc):
                            act_group(sb_plhs, j, sb_xrhs, g * gc, gc)
                    slot = 0
                    for g in range(4):
                        for _t in range(T1_COUNTS[g]):
                            for h in range(2048 // gc):
                                act_group(sb_pslhs, slot, sb_prhs,
                                          g * 2048 + h * gc, gc)
                            slot += 1

            nc.sync.dma_start(acc_d[:], sb_acc[:])

    nc.compile()
    return nc


def _plan(core):
    """Per accum column, ("t2", 1.0) or ("t1", pair_weight), in emission
    order of the dve_split program."""
    cols = []
    for j in range(NP // 128):
        cols += [("t2", 1.0), ("t2", 1.0)]          # two ACT cross groups
        if j % 4 == 3:                               # 8 DVE groups -> 1 batch col
            cols.append(("t2", 1.0))
    for _r, _g, w in _t1_pairs(core):
        cols += [("t1", w), ("t1", w)]
    return cols


def _split_hi_lo(v):
    vh = v.astype(bf16)
    vl = (v - vh.astype(np.float32)).astype(bf16)
    return vh, vl


def _enc_lhsT(p):
    """p: [n, 16] f32 -> [K, n] bf16 stationary-side encoding."""
    n = p.shape[0]
    ph, pl = _split_hi_lo(np.ascontiguousarray(p, np.float32))
    p2 = (-GAMMA * (p.astype(np.float64) ** 2).sum(-1)).astype(np.float32)
    p2h, p2l = _split_hi_lo(p2)
    out = np.empty((K, n), bf16)
    out[0:16] = ph.T
    out[16:32] = pl.T
    out[32:48] = ph.T
    out[48:64] = pl.T
    out[64] = p2h
    out[65] = p2l
    out[66] = bf16(-GAMMA)
    out[67] = bf16(-GAMMA)
    return out


def _enc_rhs(u):
    """u: [n, 16] f32 -> [K, n] bf16 moving-side encoding."""
    n = u.shape[0]
    uh, ul = _split_hi_lo(np.ascontiguousarray(u, np.float32))
    u2 = ((u.astype(np.float64) ** 2).sum(-1)).astype(np.float32)
    u2h, u2l = _split_hi_lo(u2)
    out = np.empty((K, n), bf16)
    out[0:16] = uh.T
    out[16:32] = uh.T
    out[32:48] = ul.T
    out[48:64] = ul.T
    out[64] = bf16(1.0)
    out[65] = bf16(1.0)
    out[66] = u2h
    out[67] = u2l
    return out


# ---- DVE exp offload: exp(x) = p(x)^128, p = deg-5 fit of exp(x/128) ----
_DVE_M = 128.0


def _fit_exp_coeffs():
    lo, hi = -110.0 / _DVE_M, 0.1 / _DVE_M
    k = np.arange(4000)
    nodes = (lo + hi) / 2 + (hi - lo) / 2 * np.cos((2 * k + 1) * np.pi / (2 * len(k)))
    V = np.vander(nodes, 6, increasing=True)
    w = 1.0 / np.exp(nodes)
    c = np.linalg.lstsq(V * w[:, None], np.exp(nodes) * w, rcond=None)[0]
    return (c / (_DVE_M ** np.arange(6))).astype(np.float32)


_EXP_CT = _fit_exp_coeffs()
_dve_exp_ops = None


def _register_dve_exp_ops():
    """Define + register the 3 custom DVE ops (idempotent, in-process)."""
    global _dve_exp_ops
    if _dve_exp_ops is not None:
        return _dve_exp_ops
    from operator import add as _opadd

    import concourse.dve_ops as dom
    from concourse.dve_spec import (
        C0, C1, C2, Spec, Src0, Src1, _has_src1, lower as _dve_lower, sq,
    )
    from concourse.dve_uop import DveOpSpec

    def _sq7(v):
        s = v.astype(np.float32)
        for _ in range(7):
            s = (s * s).astype(np.float32)
        return s

    specs = [
        # h1 = (c5*x + c4)*x + c3
        ("ANT_EXP128_A", Spec(
            body=(Src0 * C0 + C1) * Src0 + C2,
            reference=lambda in0, in1, c0, c1, c2: (
                (in0.astype(np.float32) * np.float32(c0) + np.float32(c1))
                * in0 + np.float32(c2)
            ).astype(np.float32),
        )),
        # p = ((h1*x + c2)*x + c1)*x + c0   (in0 = h1, in1 = x)
        ("ANT_EXP128_B", Spec(
            body=((Src0 * Src1 + C0) * Src1 + C1) * Src1 + C2,
            reference=lambda in0, in1, c0, c1, c2: (
                ((in0.astype(np.float32) * in1 + np.float32(c0)) * in1
                 + np.float32(c1)) * in1 + np.float32(c2)
            ).astype(np.float32),
        )),
    ]
    _s = Src0
    for _ in range(7):
        _s = sq(_s)
    specs.append(
        ("ANT_EXP128_C", Spec(
            body=_s,
            accum=_opadd,
            accum_init=C0,
            reference=dom._ref_body_sum(lambda in0, in1, c0, c1, c2: _sq7(in0)),
        ))
    )

    ops = []
    for name, spec in specs:
        if name in dom._SUB_OPCODE_FOR_NAME:
            ops.append(next(o for o in dom.OPS if o.name == name))
            continue
        row = dom._CUSTOM_DVE_ROW_BASE + len(dom.OPS)
        assert row < 0x20, "custom DVE opcode rows exhausted"
        op = dom.DveOp(name, spec, subdim=False, uops_sha={})
        for ver in ("v3", "v4"):
            u = _dve_lower(spec, ver=ver)
            sha = DveOpSpec(
                name=name, opcode=row, uops=u, rd1_en=_has_src1(spec)
            ).sha(ver)
            op.uops_sha[ver] = sha
        dom.OPS.append(op)
        dom._SUB_OPCODE_FOR_NAME[name] = row
        dom.CUSTOM_DVE_SPECS[name] = spec
        ops.append(op)
    _dve_exp_ops = tuple(ops)
    return _dve_exp_ops


_lock = threading.Lock()
_cached_nc = None


def _get_nc():
    global _cached_nc
    with _lock:
        if _cached_nc is None:
            _cached_nc = _build_nc()
        return _cached_nc


def _make_in_maps(x, particles):
    plhs = _enc_lhsT(particles)
    prhs = _enc_rhs(particles)
    in_maps = []
    for c in range(N_CORES):
        pairs = _t1_pairs(c)
        pslhs = np.concatenate(
            [plhs[:, r * 128:(r + 1) * 128] for r, _, _ in pairs], axis=1
        )
        in_maps.append(
            {
                "plhs": plhs,
                "prhs": prhs,
                "xrhs": _enc_rhs(x[c * XS:(c + 1) * XS]),
                "pslhs": np.ascontiguousarray(pslhs),
            }
        )
    return in_maps


def _combine(results, gc=None, dve_split=True):
    t2_sum = 0.0
    t1_sum = 0.0
    if dve_split:
        for c, r in enumerate(results):
            acc = r["acc"].astype(np.float64)
            for i, (kind, w) in enumerate(_plan(c)):
                s = acc[:, i].sum()
                if kind == "t2":
                    t2_sum += s
                else:
                    t1_sum += w * s
    else:
        if gc is None:
            gc = GC
        n_cross_groups, n_groups = _n_groups(gc)
        for c, r in enumerate(results):
            acc = r["acc"].astype(np.float64)
            t2_sum += acc[:, :n_cross_groups].sum()
            w = np.array([w for _, _, w in _t1_pairs(c)
                          for _h in range(2048 // gc)], np.float64)
            t1_sum += (acc[:, n_cross_groups:].sum(axis=0) * w).sum()
    t1 = t1_sum / (float(NP) * NP)
    t2 = 2.0 * t2_sum / (float(NX) * NP)
    return np.float32(t1 - t2)


def kernel(x, particles):
    x = np.asarray(x, np.float32)
    particles = np.asarray(particles, np.float32)
    assert x.shape == (NX, D) and particles.shape == (NP, D)

    nc = _get_nc()
    in_maps = _make_in_maps(x, particles)
    res = bass_utils.run_bass_kernel_spmd(nc, in_maps, core_ids=list(range(N_CORES)))
    return _combine(res.results)
